# revision 1
# baseline (speedup 1.0000x reference)
"""Causal self-attention (SEQ=8192, D=1024) on 8 TRN2 NeuronCores.

Strategy (SPMD, one static graph on all 8 cores):
  - Sequence parallel over queries with stride-8 row interleaving:
    core i owns query rows {8j+i : j in [0,1024)}. This balances causal
    work exactly while keeping the instruction graph identical across
    cores (per-core differences are pure data: X^T slices + masks).
  - Core i computes K^T/V projections for the contiguous key shard
    [1024*i, 1024*(i+1)). K^T/V are shared via FOUR chunked AllGathers
    (K/V x key-halves), each issued as soon as its projection slice is
    done, so the collectives overlap projection + attention compute.
  - Attention runs in S^T layout ([keys x queries]): S^T = K^T.T @ Q^T,
    so softmax(P)^T is directly the lhsT for P@V -- no transposes.
    It is split into two passes over key-halves; pass 0 only needs the
    first two gathered chunks. exp on ScalarE (scale fused), no
    max-subtraction (scores are N(0,1)-scaled), denominator via a
    ones-column matmul accumulated alongside O in PSUM.
  - All matmul operands bf16 (1 cyc/row on the PE), accumulation fp32.
"""
import sys

sys.path.insert(0, "/opt/trn_rl_repo")

import numpy as np
import ml_dtypes

import concourse.bacc as bacc
import concourse.mybir as mybir
import concourse.tile as tile
from concourse import bass_utils

S, D, NC = 8192, 1024, 8
QPC = S // NC  # 1024 queries (and kv rows) per core
NCH = D // 128  # 8 chunks of the feature dim
NQT = QPC // 128  # 8 query tiles per core
SCALE = 1.0 / np.sqrt(D).astype(np.float32)  # 1/32
BF16 = mybir.dt.bfloat16
F32 = mybir.dt.float32

_cache = {}


def _build():
    if "nc" in _cache:
        return _cache["nc"]
    nc = bacc.Bacc("TRN2", target_bir_lowering=False, debug=False, num_devices=NC)

    xt_kv = nc.dram_tensor("xt_kv", [D, QPC], BF16, kind="ExternalInput")
    xt_q = nc.dram_tensor("xt_q", [D, QPC], BF16, kind="ExternalInput")
    wkT = nc.dram_tensor("wkT", [D, D], BF16, kind="ExternalInput")
    wvT = nc.dram_tensor("wvT", [D, D], BF16, kind="ExternalInput")
    masks = nc.dram_tensor("masks", [8, 128, 128], BF16, kind="ExternalInput")
    out = nc.dram_tensor("out", [QPC, D], F32, kind="ExternalOutput")

    rg = [list(range(NC))]

    with tile.TileContext(nc) as tc:
        with tc.tile_pool(name="dram", bufs=1, space="DRAM") as dram:
            # chunked AllGather bounce buffers: K^T key-halves, V key-halves
            ag_k = [dram.tile([D, 512], BF16, name=f"agk{h}") for h in range(2)]
            ag_v = [dram.tile([512, D], BF16, name=f"agv{h}") for h in range(2)]
            g_k = [
                dram.tile([NC, D, 512], BF16, addr_space="Shared", name=f"gk{h}")
                for h in range(2)
            ]
            g_v = [
                dram.tile([NC, 512, D], BF16, addr_space="Shared", name=f"gv{h}")
                for h in range(2)
            ]

            with (
                tc.tile_pool(name="persist", bufs=1) as persist,
                tc.tile_pool(name="fin", bufs=2) as fin,
            ):
                sb_qt = persist.tile([128, NCH * QPC], BF16, tag="qt")
                sb_mask = persist.tile([128, 8 * 128], BF16, tag="msk")
                sb_ones = persist.tile([128, 1], BF16, tag="ones")
                nc.vector.memset(sb_ones[:], 1.0)

                # kv streaming pool allocated BEFORE io so its tiles
                # never alias io's SBUF (avoids WAR stalls on QT's reads)
                kv_cm = tc.tile_pool(name="kvk", bufs=8)
                kv = kv_cm.__enter__()
                kvv_cm = tc.tile_pool(name="kvv", bufs=4)
                kvv = kvv_cm.__enter__()

                # ---- projection phase ----
                with (
                    tc.tile_pool(name="io", bufs=1) as io,
                    tc.tile_pool(name="pp", bufs=4, space="PSUM") as pp,
                    tc.tile_pool(name="stage", bufs=4) as stage,
                ):
                    sb_xkv = io.tile([128, NCH * QPC], BF16, tag="xkv")
                    sb_wk = io.tile([128, NCH * D], BF16, tag="wk")
                    sb_wv = io.tile([128, NCH * D], BF16, tag="wv")
                    # consolidated input loads (one strided DMA each), K-h0
                    # critical path (wk + xkv-h0) first
                    def load_chunked(dst, src, cols):
                        nc.sync.dma_start(
                            dst.rearrange("p (c k) -> p c k", c=NCH)[:, :, 0:cols],
                            src.rearrange("(c p) k -> p c k", p=128),
                        )

                    # sync-queue FIFO order doubles as DMA priority
                    load_chunked(sb_xkv, xt_kv[:, 0:512], 512)
                    nc.sync.dma_start(
                        sb_wk.rearrange("p (c k) -> p c k", c=NCH)[:, :, 0:512],
                        wkT[:, 0:512].rearrange("(c p) k -> p c k", p=128),
                    )
                    nc.sync.dma_start(
                        sb_wk.rearrange("p (c k) -> p c k", c=NCH)[:, :, 512:1024],
                        wkT[:, 512:1024].rearrange("(c p) k -> p c k", p=128),
                    )
                    nc.sync.dma_start(
                        sb_mask.rearrange("k (t q) -> k t q", t=8),
                        masks.rearrange("t k q -> k t q"),
                    )
                    load_chunked(sb_wv, wvT, D)
                    nc.sync.dma_start(
                        sb_xkv.rearrange("p (c k) -> p c k", c=NCH)[:, :, 512:1024],
                        xt_kv[:, 512:1024].rearrange("(c p) k -> p c k", p=128),
                    )

                    def proj_group(lhs_sb, lhs_off, rhs_sb, rhs_off):
                        """8-chunk contraction matmul into a fresh PSUM tile.

                        All projection SBUF tiles share the layout
                        [128, 8*1024]: in-dim chunk c at cols [c*1024, ...).
                        """
                        ps = pp.tile([128, 512], F32, tag="pp", name="ps")
                        for c in range(NCH):
                            nc.tensor.matmul(
                                ps[:],
                                lhs_sb[:, c * 1024 + lhs_off : c * 1024 + lhs_off + 128],
                                rhs_sb[:, c * 1024 + rhs_off : c * 1024 + rhs_off + 512],
                                start=(c == 0),
                                stop=(c == NCH - 1),
                            )
                        return ps

                    def proj_group2(lhs_sb, lhs_off, rhs_sb, rhs_off0, rhs_off1):
                        """Two 512-wide outputs sharing the stationary operand
                        (back-to-back matmuls reuse the loaded weights)."""
                        ps0 = pp.tile([128, 512], F32, tag="pp", name="ps0")
                        ps1 = pp.tile([128, 512], F32, tag="pp", name="ps1")
                        for c in range(NCH):
                            lhs = lhs_sb[
                                :, c * 1024 + lhs_off : c * 1024 + lhs_off + 128
                            ]
                            nc.tensor.matmul(
                                ps0[:],
                                lhs,
                                rhs_sb[:, c * 1024 + rhs_off0 : c * 1024 + rhs_off0 + 512],
                                start=(c == 0),
                                stop=(c == NCH - 1),
                            )
                            nc.tensor.matmul(
                                ps1[:],
                                lhs,
                                rhs_sb[:, c * 1024 + rhs_off1 : c * 1024 + rhs_off1 + 512],
                                start=(c == 0),
                                stop=(c == NCH - 1),
                            )
                        return ps0, ps1


                    # K^T key-half h: rows = out-dim chunks oc, cols keys
                    # [512h, 512h+512); then V key-half h: key chunks kc.
                    for h in range(2):
                        for oc in range(NCH):
                            ps = proj_group(sb_wk, oc * 128, sb_xkv, h * 512)
                            stg = stage.tile([128, 512], BF16, tag="stg", name="stg")
                            nc.any.tensor_copy(stg[:], ps[:])
                            nc.sync.dma_start(
                                ag_k[h][oc * 128 : (oc + 1) * 128, :], stg[:]
                            )
                        nc.gpsimd.collective_compute(
                            "AllGather",
                            mybir.AluOpType.bypass,
                            replica_groups=rg,
                            ins=[ag_k[h].opt()],
                            outs=[g_k[h].opt()],
                        )
                        if h == 0:
                            load_chunked(sb_qt, xt_q, QPC)
                        for kc4 in range(4):
                            kc = h * 4 + kc4
                            ps0, ps1 = proj_group2(sb_xkv, kc * 128, sb_wv, 0, 512)
                            for dh, ps in ((0, ps0), (1, ps1)):
                                stg = stage.tile(
                                    [128, 512], BF16, tag="stg", name="stg"
                                )
                                nc.any.tensor_copy(stg[:], ps[:])
                                nc.sync.dma_start(
                                    ag_v[h][
                                        kc4 * 128 : (kc4 + 1) * 128,
                                        dh * 512 : (dh + 1) * 512,
                                    ],
                                    stg[:],
                                )
                        nc.gpsimd.collective_compute(
                            "AllGather",
                            mybir.AluOpType.bypass,
                            replica_groups=rg,
                            ins=[ag_v[h].opt()],
                            outs=[g_v[h].opt()],
                        )

                # ---- attention: two passes over key-halves, each pass split
                # into an ST phase (needs only gathered K^T) and a PV phase
                # (needs gathered V) so collectives hide behind compute ----
                with (
                    tc.tile_pool(name="oacc", bufs=1) as oaccp,
                    tc.tile_pool(name="psst", bufs=2, space="PSUM") as psst,
                    tc.tile_pool(name="pso", bufs=2, space="PSUM") as pso,
                ):
                    o_acc = [
                        oaccp.tile([128, D + 1], F32, tag=f"oacc{j}", name=f"oacc{j}")
                        for j in range(NQT)
                    ]

                    def j_groups(Sb):
                        """Contiguous J-tile ranges covering J in [Sb, 8)."""
                        if Sb + 4 < NQT:
                            return [(Sb, Sb + 4), (Sb + 4, NQT)]
                        return [(Sb, NQT)]

                    for H in range(2):
                        with tc.tile_pool(name=f"ptp{H}", bufs=1) as ptp:
                            pts = {}
                            # -- ST block: S^T = K^T.T @ Q^T, exp, mask --
                            def st_block(Sb):
                                    kt_t = kv.tile(
                                        [128, NCH * 512], BF16, tag="kt", name="kt_t"
                                    )
                                    for cp in range(NCH):
                                        eng = nc.gpsimd if cp == 7 else nc.sync
                                        eng.dma_start(
                                            kt_t[:, cp * 512 : (cp + 1) * 512],
                                            g_k[H][Sb, 128 * cp : 128 * (cp + 1), :],
                                        )
                                    for kt4 in range(4):
                                        kt = H * 4 + kt4
                                        for (j0, j1) in j_groups(Sb):
                                            N = (j1 - j0) * 128
                                            # Diagonal group: queries below q0 are
                                            # fully masked for this key tile (for
                                            # every core: 128*kt > 8*q+7), so skip
                                            # their ST columns.  exp reads stale
                                            # PSUM there (finite) and the mask
                                            # multiply zeroes it.
                                            q0 = max(0, 16 * kt - 1) if j0 == Sb else 0
                                            st = psst.tile(
                                                [128, 512], F32, tag="st", name="st"
                                            )
                                            for c in range(NCH):
                                                nc.tensor.matmul(
                                                    st[:, q0:N],
                                                    kt_t[
                                                        :,
                                                        c * 512
                                                        + kt4 * 128 : c * 512
                                                        + kt4 * 128
                                                        + 128,
                                                    ],
                                                    sb_qt[
                                                        :,
                                                        c * QPC
                                                        + j0 * 128
                                                        + q0 : c * QPC
                                                        + j1 * 128,
                                                    ],
                                                    start=(c == 0),
                                                    stop=(c == NCH - 1),
                                                )
                                            pt = ptp.tile(
                                                [128, N],
                                                BF16,
                                                tag=f"pt{Sb}_{j0}_{kt4}",
                                                name=f"pt{Sb}_{j0}_{kt4}",
                                            )
                                            nc.scalar.activation(
                                                pt[:],
                                                st[:, 0:N],
                                                mybir.ActivationFunctionType.Exp,
                                                scale=float(SCALE),
                                            )
                                            if j0 == Sb:
                                                # first J-tile of the group is the
                                                # causal diagonal -> mask it
                                                nc.vector.tensor_mul(
                                                    pt[:, 0:128],
                                                    pt[:, 0:128],
                                                    sb_mask[:, kt * 128 : kt * 128 + 128],
                                                )
                                            pts[(Sb, j0, kt4)] = pt

                            # -- PV block: O += P^T.T @ V, denom via ones --
                            def pv_block(Sb):
                                    v_t = kvv.tile([128, 4 * D], BF16, tag="v", name="v_t")
                                    for cp in range(4):
                                        nc.sync.dma_start(
                                            v_t[:, cp * D : (cp + 1) * D],
                                            g_v[H][Sb, 128 * cp : 128 * (cp + 1), :],
                                        )
                                    for J in range(Sb, NQT):
                                        j0 = Sb if J < min(Sb + 4, NQT) else Sb + 4
                                        o_ps = pso.tile(
                                            [128, 1536], F32, tag="ops", name="o_ps"
                                        )
                                        for kt4 in range(4):
                                            pt = pts[(Sb, j0, kt4)]
                                            lhsT = pt[:, (J - j0) * 128 : (J - j0 + 1) * 128]
                                            nc.tensor.matmul(
                                                o_ps[:, 0:512],
                                                lhsT,
                                                v_t[:, kt4 * D : kt4 * D + 512],
                                                start=(kt4 == 0),
                                                stop=(kt4 == 3),
                                            )
                                            nc.tensor.matmul(
                                                o_ps[:, 512:1024],
                                                lhsT,
                                                v_t[:, kt4 * D + 512 : kt4 * D + 1024],
                                                start=(kt4 == 0),
                                                stop=(kt4 == 3),
                                            )
                                            nc.tensor.matmul(
                                                o_ps[:, 1024:1025],
                                                lhsT,
                                                sb_ones[:],
                                                start=(kt4 == 0),
                                                stop=(kt4 == 3),
                                            )

                                        if H == 0 and Sb == 0:
                                            nc.vector.tensor_copy(
                                                o_acc[J][:], o_ps[:, 0 : D + 1]
                                            )
                                        else:
                                            nc.vector.tensor_add(
                                                o_acc[J][:], o_acc[J][:], o_ps[:, 0 : D + 1]
                                            )

                                        if H == 1 and Sb == J:
                                            rs = fin.tile([128, 1], F32, tag="rs", name="rs")
                                            nc.vector.reciprocal(
                                                rs[:], o_acc[J][:, D : D + 1]
                                            )
                                            outt = fin.tile(
                                                [128, D], F32, tag="outt", name="outt"
                                            )
                                            nc.vector.tensor_scalar_mul(
                                                outt[:], o_acc[J][:, 0:D], rs[:]
                                            )
                                            nc.sync.dma_start(
                                                out[J * 128 : (J + 1) * 128, :], outt[:]
                                            )

                            if H == 0:
                                # interleave aligned with stream arrival: PE
                                # reaches pv(0) at ~K1+58us, V1 lands at
                                # ~K1+50us, and the sync-ring FIFO order
                                # (kt0-3, v0, kt4, v1, ...) matches the
                                # consumption order with slack at each step
                                for Sb in range(4):
                                    st_block(Sb)
                                pv_block(0)
                                st_block(4)
                                pv_block(1)
                                st_block(5)
                                pv_block(2)
                                st_block(6)
                                pv_block(3)
                                st_block(7)
                                for Sb in range(4, NC):
                                    pv_block(Sb)
                            else:
                                for Sb in range(NC):
                                    st_block(Sb)
                                for Sb in range(NC):
                                    pv_block(Sb)
                kvv_cm.__exit__(None, None, None)
                kv_cm.__exit__(None, None, None)

    nc.compile()
    _cache["nc"] = nc
    return nc


def _make_in_maps(inputs, w_query, w_key, w_value):
    bf = ml_dtypes.bfloat16
    xt = np.ascontiguousarray(inputs.T.astype(np.float32))  # [D, S]
    # Wq absorbed into the key path: scores = x_k^T (Wk^T Wq) x_q
    wkT = np.ascontiguousarray(
        w_key.T.astype(np.float32) @ w_query.astype(np.float32)
    ).astype(bf)
    wvT = np.ascontiguousarray(w_value.T).astype(bf)

    kt_off = np.arange(8)[:, None, None] * 128 + np.arange(128)[None, :, None]
    in_maps = []
    for i in range(NC):
        xkv = np.ascontiguousarray(xt[:, i * QPC : (i + 1) * QPC]).astype(bf)
        xq = np.ascontiguousarray(xt[:, i::NC]).astype(bf)
        q_off = np.arange(128)[None, None, :] * 8 + i
        m = (kt_off <= q_off).astype(np.float32).astype(bf)  # [8,128,128]
        in_maps.append(
            {
                "xt_kv": xkv,
                "xt_q": xq,
                "wkT": wkT,
                "wvT": wvT,
                "masks": np.ascontiguousarray(m),
            }
        )
    return in_maps


def run(inputs, w_query, w_key, w_value, trace=False):
    nc = _build()
    in_maps = _make_in_maps(inputs, w_query, w_key, w_value)
    res = bass_utils.run_bass_kernel_spmd(
        nc, in_maps, core_ids=list(range(NC)), trace=trace
    )
    full = np.empty((S, D), dtype=np.float32)
    for i in range(NC):
        full[i::NC] = res.results[i]["out"]
    return full, res


def kernel(inputs, w_query, w_key, w_value):
    inputs = np.asarray(inputs, dtype=np.float32)
    w_query = np.asarray(w_query, dtype=np.float32)
    w_key = np.asarray(w_key, dtype=np.float32)
    w_value = np.asarray(w_value, dtype=np.float32)
    full, _ = run(inputs, w_query, w_key, w_value, trace=False)
    return full



# revision 4
# speedup vs baseline: 1.3054x; 1.3054x over previous
"""Causal self-attention (SEQ=8192, D=1024) on 8 TRN2 NeuronCores.

Strategy (SPMD, one static graph on all 8 cores, ZERO collectives):
  - scores = x_q (Wq^T Wk) x_k^T: fold the combined weight M = Wq^T Wk
    into the QUERY side. Each core projects only its own 1024 strided
    queries (Q'' = M^T x_q^T, 27us of PE); the keys are the RAW input
    x^T, replicated to every core's HBM by the host and streamed from
    local DRAM. No K AllGather.
  - O = (P @ x) Wv^T: apply Wv on the OUTPUT side. PV accumulates
    O1 = P^T.T @ x_chunk (raw x as "values", again local HBM), and a
    final per-core [1024q x 1024] @ Wv^T projection replaces the
    sharded V projection at identical FLOP cost. No V AllGather.
  - Sequence-parallel over queries with stride-8 row interleaving
    (core i owns query rows {8j+i}) exactly balances causal work while
    keeping one SPMD graph; per-core differences are data only
    (x_q^T slice + causal masks).
  - Attention runs in S^T layout over 16 key blocks of 512: S^T chunk =
    x_k-chunk^T.T @ Q''^T, exp on ScalarE (scale fused), diagonal-block
    masking by a data mask, denominator via a ones column baked into
    the streamed x (ones-matmul accumulated alongside O1 in PSUM).
  - Output: O1 (f32) -> bf16 -> XBAR DMA-transpose (free, on DMA
    engines) -> 16 matmuls vs Wv^T chunks -> scale by 1/denominator.
    The out-projection matmuls for J are deferred one key-block so the
    transpose latency never blocks the Tensor queue.
  - All matmul operands bf16 (1 cyc/row on the PE), accumulation fp32.
"""
import sys

sys.path.insert(0, "/opt/trn_rl_repo")

import numpy as np
import ml_dtypes

import concourse.bacc as bacc
import concourse.mybir as mybir
import concourse.tile as tile
from concourse import bass_utils

S, D, NC = 8192, 1024, 8
QPC = S // NC  # 1024 queries per core
NCH = D // 128  # 8 chunks of the feature dim
NQT = QPC // 128  # 8 query tiles (J) per core
NKB = S // 512  # 16 key blocks of 512
XFW = 1028  # xf row width: 1024 feats + ones col + pad
SCALE = 1.0 / np.sqrt(D).astype(np.float32)  # 1/32
BF16 = mybir.dt.bfloat16
F32 = mybir.dt.float32

_cache = {}


def _build():
    if "nc" in _cache:
        return _cache["nc"]
    nc = bacc.Bacc("TRN2", target_bir_lowering=False, debug=False, num_devices=NC)

    m_in = nc.dram_tensor("m", [D, D], BF16, kind="ExternalInput")
    xq_in = nc.dram_tensor("xq", [D, QPC], BF16, kind="ExternalInput")
    xt_in = nc.dram_tensor("xt", [D, S], BF16, kind="ExternalInput")
    xf_in = nc.dram_tensor("xf", [S, XFW], BF16, kind="ExternalInput")
    wvt_in = nc.dram_tensor("wvt", [D, D], BF16, kind="ExternalInput")
    masks = nc.dram_tensor("masks", [8, 128, 128], BF16, kind="ExternalInput")
    out = nc.dram_tensor("out", [QPC, D], F32, kind="ExternalOutput")

    with tile.TileContext(nc) as tc:
        with (
            tc.tile_pool(name="persist", bufs=1) as persist,
            tc.tile_pool(name="kv", bufs=3) as kv,
            tc.tile_pool(name="kvv", bufs=3) as kvv,
            tc.tile_pool(name="ptp", bufs=2) as ptp,
            tc.tile_pool(name="fin", bufs=2) as fin,
        ):
            sb_qt = persist.tile([128, NCH * QPC], BF16, tag="qt")
            sb_wv = persist.tile([128, NCH * D], BF16, tag="wv")
            sb_mask = persist.tile([128, 8 * 128], BF16, tag="msk")
            o_acc = [
                persist.tile([128, D + 1], F32, tag=f"oacc{j}", name=f"oacc{j}")
                for j in range(NQT)
            ]

            def load_chunked(dst, src, cols):
                nc.sync.dma_start(
                    dst.rearrange("p (c k) -> p c k", c=NCH)[:, :, 0:cols],
                    src.rearrange("(c p) k -> p c k", p=128),
                )

            # ---- Q'' projection: sb_qt = (M^T x_q^T) chunks ----
            with (
                tc.tile_pool(name="io", bufs=1) as io,
                tc.tile_pool(name="pp", bufs=4, space="PSUM") as pp,
            ):
                sb_m = io.tile([128, NCH * D], BF16, tag="m")
                sb_xq = io.tile([128, NCH * QPC], BF16, tag="xq")
                load_chunked(sb_m, m_in, D)
                load_chunked(sb_xq, xq_in, QPC)
                nc.sync.dma_start(
                    sb_mask.rearrange("k (t q) -> k t q", t=8),
                    masks.rearrange("t k q -> k t q"),
                )
                load_chunked(sb_wv, wvt_in, D)

                for fo in range(NCH):
                    for half in range(2):
                        ps = pp.tile([128, 512], F32, tag="pp", name="ps")
                        for c in range(NCH):
                            nc.tensor.matmul(
                                ps[:],
                                sb_m[:, c * D + fo * 128 : c * D + fo * 128 + 128],
                                sb_xq[
                                    :,
                                    c * QPC + half * 512 : c * QPC + half * 512 + 512,
                                ],
                                start=(c == 0),
                                stop=(c == NCH - 1),
                            )
                        nc.any.tensor_copy(
                            sb_qt[
                                :,
                                fo * QPC + half * 512 : fo * QPC + half * 512 + 512,
                            ],
                            ps[:],
                        )

            # ---- attention over 16 key blocks of 512 ----
            with (
                tc.tile_pool(name="psst", bufs=2, space="PSUM") as psst,
                tc.tile_pool(name="pso", bufs=2, space="PSUM") as pso,
            ):

                def j_groups(Sb):
                    if Sb + 4 < NQT:
                        return [(Sb, Sb + 4), (Sb + 4, NQT)]
                    return [(Sb, NQT)]

                def st_block(kb):
                    Sb, H = kb >> 1, kb & 1
                    kt_t = kv.tile([128, NCH * 512], BF16, tag="kt", name="kt_t")
                    nc.sync.dma_start(
                        kt_t.rearrange("p (c k) -> p c k", c=NCH),
                        xt_in.rearrange("(c p) k -> p c k", p=128)[
                            :, :, kb * 512 : (kb + 1) * 512
                        ],
                    )
                    pts = {}
                    for kt4 in range(4):
                        kt = H * 4 + kt4
                        for gi, (j0, j1) in enumerate(j_groups(Sb)):
                            N = (j1 - j0) * 128
                            q0 = max(0, 16 * kt - 1) if j0 == Sb else 0
                            st = psst.tile([128, 512], F32, tag="st", name="st")
                            for c in range(NCH):
                                nc.tensor.matmul(
                                    st[:, q0:N],
                                    kt_t[
                                        :,
                                        c * 512 + kt4 * 128 : c * 512 + kt4 * 128 + 128,
                                    ],
                                    sb_qt[
                                        :,
                                        c * QPC + j0 * 128 + q0 : c * QPC + j1 * 128,
                                    ],
                                    start=(c == 0),
                                    stop=(c == NCH - 1),
                                )
                            pt = ptp.tile(
                                [128, 512],
                                BF16,
                                tag=f"pt{gi}_{kt4}",
                                name=f"pt{gi}_{kt4}",
                            )
                            nc.scalar.activation(
                                pt[:, 0:N],
                                st[:, 0:N],
                                mybir.ActivationFunctionType.Exp,
                                scale=float(SCALE),
                            )
                            if j0 == Sb:
                                nc.vector.tensor_mul(
                                    pt[:, 0:128],
                                    pt[:, 0:128],
                                    sb_mask[:, kt * 128 : kt * 128 + 128],
                                )
                            pts[(gi, kt4)] = pt
                    return pts

                def finalize_front(J):
                    """o_acc[J] complete: reciprocal + bf16 cast + XBAR
                    transpose. PE-side projection deferred (outproj_pe)."""
                    rec = fin.tile([128, 1], F32, tag="rec", name="rec")
                    nc.vector.reciprocal(rec[:], o_acc[J][:, D : D + 1])
                    o1 = fin.tile([128, D], BF16, tag="o1", name="o1")
                    nc.vector.tensor_copy(o1[:], o_acc[J][:, 0:D])
                    o1t = fin.tile([128, NCH * 128], BF16, tag="o1t", name="o1t")
                    nc.scalar.dma_start_transpose(
                        o1t.rearrange("p (c q) -> p c q", c=NCH),
                        o1[:],
                    )
                    return rec, o1t

                def outproj_pe(J, rec, o1t):
                    out_ps = pso.tile([128, 1536], F32, tag="ops", name="out_ps")
                    o1t3 = o1t.rearrange("p (c q) -> p c q", c=NCH)
                    for half in range(2):
                        for c in range(NCH):
                            nc.tensor.matmul(
                                out_ps[:, half * 512 : half * 512 + 512],
                                o1t3[:, c, :],
                                sb_wv[
                                    :,
                                    c * D + half * 512 : c * D + half * 512 + 512,
                                ],
                                start=(c == 0),
                                stop=(c == NCH - 1),
                            )
                    outt = fin.tile([128, D], F32, tag="outt", name="outt")
                    nc.vector.tensor_scalar_mul(outt[:], out_ps[:, 0:D], rec[:])
                    nc.gpsimd.dma_start(out[J * 128 : (J + 1) * 128, :], outt[:])

                def pv_block(kb, pts):
                    Sb = kb >> 1
                    v_t = kvv.tile([128, 4 * XFW], BF16, tag="v", name="v_t")
                    nc.sync.dma_start(
                        v_t.rearrange("p (c w) -> p c w", c=4),
                        xf_in[kb * 512 : (kb + 1) * 512, :].rearrange(
                            "(c p) w -> p c w", p=128
                        ),
                    )
                    done = None
                    for J in range(Sb, NQT):
                        gi = 0 if J < min(Sb + 4, NQT) else 1
                        j0 = Sb if gi == 0 else Sb + 4
                        o_ps = pso.tile([128, 1536], F32, tag="ops", name="o_ps")
                        for kt4 in range(4):
                            pt = pts[(gi, kt4)]
                            lhsT = pt[:, (J - j0) * 128 : (J - j0 + 1) * 128]
                            for dh in range(2):
                                nc.tensor.matmul(
                                    o_ps[:, dh * 512 : dh * 512 + 512],
                                    lhsT,
                                    v_t[
                                        :,
                                        kt4 * XFW + dh * 512 : kt4 * XFW + dh * 512 + 512,
                                    ],
                                    start=(kt4 == 0),
                                    stop=(kt4 == 3),
                                )
                            nc.tensor.matmul(
                                o_ps[:, 1024:1025],
                                lhsT,
                                v_t[:, kt4 * XFW + 1024 : kt4 * XFW + 1025],
                                start=(kt4 == 0),
                                stop=(kt4 == 3),
                            )
                        if kb == 0:
                            nc.vector.tensor_copy(o_acc[J][:], o_ps[:, 0 : D + 1])
                        else:
                            nc.vector.tensor_add(
                                o_acc[J][:], o_acc[J][:], o_ps[:, 0 : D + 1]
                            )
                        if kb == 2 * J + 1:
                            done = (J, *finalize_front(J))
                    return done

                pending = None
                for kb in range(NKB):
                    pts = st_block(kb)
                    if pending is not None:
                        outproj_pe(*pending)
                        pending = None
                    done = pv_block(kb, pts)
                    if done is not None:
                        J, rec, o1t = done
                        if J < NQT - 1:
                            pending = (J, rec, o1t)
                        else:
                            outproj_pe(J, rec, o1t)

    nc.compile()
    _cache["nc"] = nc
    return nc


def _make_in_maps(inputs, w_query, w_key, w_value):
    bf = ml_dtypes.bfloat16
    x32 = inputs.astype(np.float32)
    xt = np.ascontiguousarray(x32.T).astype(bf)  # [D, S]
    xf = np.zeros((S, XFW), dtype=bf)
    xf[:, 0:D] = x32.astype(bf)
    xf[:, D] = np.float32(1.0)
    # M = Wq^T Wk folded into the query side
    m = np.ascontiguousarray(
        w_query.astype(np.float32).T @ w_key.astype(np.float32)
    ).astype(bf)
    wvt = np.ascontiguousarray(w_value.T).astype(bf)

    kt_off = np.arange(8)[:, None, None] * 128 + np.arange(128)[None, :, None]
    in_maps = []
    for i in range(NC):
        xq = np.ascontiguousarray(xt[:, i::NC])
        q_off = np.arange(128)[None, None, :] * 8 + i
        mask = (kt_off <= q_off).astype(np.float32).astype(bf)  # [8,128,128]
        in_maps.append(
            {
                "m": m,
                "xq": xq,
                "xt": xt,
                "xf": xf,
                "wvt": wvt,
                "masks": np.ascontiguousarray(mask),
            }
        )
    return in_maps


def run(inputs, w_query, w_key, w_value, trace=False):
    nc = _build()
    in_maps = _make_in_maps(inputs, w_query, w_key, w_value)
    res = bass_utils.run_bass_kernel_spmd(
        nc, in_maps, core_ids=list(range(NC)), trace=trace
    )
    full = np.empty((S, D), dtype=np.float32)
    for i in range(NC):
        full[i::NC] = res.results[i]["out"]
    return full, res


def kernel(inputs, w_query, w_key, w_value):
    inputs = np.asarray(inputs, dtype=np.float32)
    w_query = np.asarray(w_query, dtype=np.float32)
    w_key = np.asarray(w_key, dtype=np.float32)
    w_value = np.asarray(w_value, dtype=np.float32)
    full, _ = run(inputs, w_query, w_key, w_value, trace=False)
    return full


# revision 5
# speedup vs baseline: 1.3611x; 1.0427x over previous
"""Causal self-attention (SEQ=8192, D=1024) on 8 TRN2 NeuronCores.

Strategy (SPMD, one static graph on all 8 cores, ZERO collectives):
  - scores = x_q (Wq^T Wk) x_k^T: fold the combined weight M = Wq^T Wk
    into the QUERY side. Each core projects only its own 1024 strided
    queries (Q''^T = M^T x_q^T, ~27us of PE); the keys are the RAW
    input x^T, replicated to every core's HBM by the host and streamed
    from local DRAM. No K AllGather.
  - O = (P @ x) Wv^T: apply Wv on the OUTPUT side. PV accumulates
    O1 = P^T.T @ x_chunk (raw x as "values", again local HBM), and a
    final per-core [1024q x 1024] @ Wv^T projection replaces the
    sharded V projection at identical FLOP cost. No V AllGather.
  - Sequence-parallel over queries with stride-8 row interleaving
    (core i owns query rows {8j+i}) exactly balances causal work while
    keeping one SPMD graph; per-core differences are data only
    (x_q^T slice + causal masks).
  - Attention runs in S^T layout over 16 key blocks of 512: S^T chunk =
    x_k-chunk^T.T @ Q''^T, exp on ScalarE (scale fused), diagonal-block
    masking by a data mask, denominator via a ones column baked into
    the streamed x (ones-matmul accumulated alongside O1 in PSUM).
  - Output: O1 (f32) -> bf16 -> XBAR DMA-transpose (free, on DMA
    engines) -> 16 matmuls vs Wv^T chunks -> scale by 1/denominator.
    The out-projection for J is deferred by two key blocks so the
    add/cast/transpose latency never blocks the Tensor queue; J=6's
    runs after PV(15) to keep the PE warm through J=7's finalize.
  - All inputs are host-pre-arranged so every DMA is a [128, N]
    contiguous transfer (128 descriptors, no descriptor storms).
  - All matmul operands bf16 (1 cyc/row on the PE), accumulation fp32.
"""
import sys

sys.path.insert(0, "/opt/trn_rl_repo")

import numpy as np
import ml_dtypes

import concourse.bacc as bacc
import concourse.mybir as mybir
import concourse.tile as tile
from concourse import bass_utils

S, D, NC = 8192, 1024, 8
QPC = S // NC  # 1024 queries per core
NCH = D // 128  # 8 chunks of the feature dim
NQT = QPC // 128  # 8 query tiles (J) per core
NKB = S // 512  # 16 key blocks of 512
XFW = 1028  # xf row width: 1024 feats + ones col + pad
SCALE = 1.0 / np.sqrt(D).astype(np.float32)  # 1/32
BF16 = mybir.dt.bfloat16
F32 = mybir.dt.float32

_cache = {}


def _build():
    if "nc" in _cache:
        return _cache["nc"]
    nc = bacc.Bacc("TRN2", target_bir_lowering=False, debug=False, num_devices=NC)

    # all pre-arranged on host: partition dim first, contiguous free dim
    m_in = nc.dram_tensor("m", [128, NCH * D], BF16, kind="ExternalInput")
    xq0_in = nc.dram_tensor("xq0", [128, NCH * 512], BF16, kind="ExternalInput")
    xq1_in = nc.dram_tensor("xq1", [128, NCH * 512], BF16, kind="ExternalInput")
    xt_in = nc.dram_tensor("xt", [NKB, 128, NCH * 512], BF16, kind="ExternalInput")
    xf_in = nc.dram_tensor("xf", [NKB, 128, 4 * XFW], BF16, kind="ExternalInput")
    wv_in = nc.dram_tensor("wv", [128, NCH * D], BF16, kind="ExternalInput")
    mask_in = nc.dram_tensor("masks", [128, 8 * 128], BF16, kind="ExternalInput")
    out = nc.dram_tensor("out", [QPC, D], F32, kind="ExternalOutput")

    with tile.TileContext(nc) as tc:
        with (
            tc.tile_pool(name="persist", bufs=1) as persist,
            tc.tile_pool(name="kv", bufs=3) as kv,
            tc.tile_pool(name="kvv", bufs=3) as kvv,
            tc.tile_pool(name="ptp", bufs=2) as ptp,
            tc.tile_pool(name="fin", bufs=2) as fin,
        ):
            sb_qt = persist.tile([128, NCH * QPC], BF16, tag="qt")
            sb_wv = persist.tile([128, NCH * D], BF16, tag="wv")
            sb_mask = persist.tile([128, 8 * 128], BF16, tag="msk")
            o_acc = [
                persist.tile([128, D + 1], F32, tag=f"oacc{j}", name=f"oacc{j}")
                for j in range(NQT)
            ]

            # ---- Q'' projection: sb_qt = (M^T x_q^T) chunks ----
            with (
                tc.tile_pool(name="io", bufs=1) as io,
                tc.tile_pool(name="pp", bufs=4, space="PSUM") as pp,
            ):
                sb_m = io.tile([128, NCH * D], BF16, tag="m")
                sb_xq = [
                    io.tile([128, NCH * 512], BF16, tag=f"xq{h}", name=f"xq{h}")
                    for h in range(2)
                ]
                nc.sync.dma_start(sb_m[:], m_in[:])
                nc.sync.dma_start(sb_xq[0][:], xq0_in[:])
                nc.sync.dma_start(sb_xq[1][:], xq1_in[:])

                for half in range(2):
                    for fo in range(NCH):
                        ps = pp.tile([128, 512], F32, tag="pp", name="ps")
                        for c in range(NCH):
                            nc.tensor.matmul(
                                ps[:],
                                sb_m[:, c * D + fo * 128 : c * D + fo * 128 + 128],
                                sb_xq[half][:, c * 512 : (c + 1) * 512],
                                start=(c == 0),
                                stop=(c == NCH - 1),
                            )
                        nc.any.tensor_copy(
                            sb_qt[
                                :,
                                fo * QPC + half * 512 : fo * QPC + half * 512 + 512,
                            ],
                            ps[:],
                        )

            # ---- attention over 16 key blocks of 512 ----
            with (
                tc.tile_pool(name="psst", bufs=2, space="PSUM") as psst,
                tc.tile_pool(name="pso", bufs=2, space="PSUM") as pso,
            ):
                nc.sync.dma_start(sb_mask[:], mask_in[:])
                nc.sync.dma_start(sb_wv[:], wv_in[:])

                def j_groups(Sb):
                    if Sb + 4 < NQT:
                        return [(Sb, Sb + 4), (Sb + 4, NQT)]
                    return [(Sb, NQT)]

                def st_block(kb):
                    Sb, H = kb >> 1, kb & 1
                    kt_t = kv.tile([128, NCH * 512], BF16, tag="kt", name="kt_t")
                    nc.sync.dma_start(kt_t[:], xt_in[kb])
                    pts = {}
                    for kt4 in range(4):
                        kt = H * 4 + kt4
                        for gi, (j0, j1) in enumerate(j_groups(Sb)):
                            N = (j1 - j0) * 128
                            q0 = max(0, 16 * kt - 1) if j0 == Sb else 0
                            st = psst.tile([128, 512], F32, tag="st", name="st")
                            for c in range(NCH):
                                nc.tensor.matmul(
                                    st[:, q0:N],
                                    kt_t[
                                        :,
                                        c * 512 + kt4 * 128 : c * 512 + kt4 * 128 + 128,
                                    ],
                                    sb_qt[
                                        :,
                                        c * QPC + j0 * 128 + q0 : c * QPC + j1 * 128,
                                    ],
                                    start=(c == 0),
                                    stop=(c == NCH - 1),
                                )
                            pt = ptp.tile(
                                [128, 512],
                                BF16,
                                tag=f"pt{gi}_{kt4}",
                                name=f"pt{gi}_{kt4}",
                            )
                            nc.scalar.activation(
                                pt[:, 0:N],
                                st[:, 0:N],
                                mybir.ActivationFunctionType.Exp,
                                scale=float(SCALE),
                            )
                            if j0 == Sb:
                                nc.vector.tensor_mul(
                                    pt[:, 0:128],
                                    pt[:, 0:128],
                                    sb_mask[:, kt * 128 : kt * 128 + 128],
                                )
                            pts[(gi, kt4)] = pt
                    return pts

                def finalize_front(J, o_ps):
                    """Last PV chunk for J: fuse the final o_acc add with the
                    bf16 cast, then reciprocal + XBAR transpose. The PE-side
                    out-projection is deferred (outproj_pe)."""
                    o1 = fin.tile([128, D], BF16, tag="o1", name="o1")
                    nc.vector.tensor_add(o1[:], o_acc[J][:, 0:D], o_ps[:, 0:D])
                    dd = fin.tile([128, 1], F32, tag="dd", name="dd")
                    nc.vector.tensor_add(
                        dd[:], o_acc[J][:, D : D + 1], o_ps[:, D : D + 1]
                    )
                    rec = fin.tile([128, 1], F32, tag="rec", name="rec")
                    nc.vector.reciprocal(rec[:], dd[:])
                    o1t = fin.tile([128, NCH * 128], BF16, tag="o1t", name="o1t")
                    nc.scalar.dma_start_transpose(
                        o1t.rearrange("p (c q) -> p c q", c=NCH),
                        o1[:],
                    )
                    return rec, o1t

                def outproj_pe(J, rec, o1t):
                    out_ps = pso.tile([128, 1536], F32, tag="ops", name="out_ps")
                    o1t3 = o1t.rearrange("p (c q) -> p c q", c=NCH)
                    for half in range(2):
                        for c in range(NCH):
                            nc.tensor.matmul(
                                out_ps[:, half * 512 : half * 512 + 512],
                                o1t3[:, c, :],
                                sb_wv[
                                    :,
                                    c * D + half * 512 : c * D + half * 512 + 512,
                                ],
                                start=(c == 0),
                                stop=(c == NCH - 1),
                            )
                    outt = fin.tile([128, D], F32, tag="outt", name="outt")
                    nc.vector.tensor_scalar_mul(outt[:], out_ps[:, 0:D], rec[:])
                    nc.gpsimd.dma_start(out[J * 128 : (J + 1) * 128, :], outt[:])

                def pv_block(kb, pts):
                    Sb = kb >> 1
                    v_t = kvv.tile([128, 4 * XFW], BF16, tag="v", name="v_t")
                    nc.sync.dma_start(v_t[:], xf_in[kb])
                    done = None
                    for J in range(Sb, NQT):
                        gi = 0 if J < min(Sb + 4, NQT) else 1
                        j0 = Sb if gi == 0 else Sb + 4
                        o_ps = pso.tile([128, 1536], F32, tag="ops", name="o_ps")
                        for kt4 in range(4):
                            pt = pts[(gi, kt4)]
                            lhsT = pt[:, (J - j0) * 128 : (J - j0 + 1) * 128]
                            for dh in range(2):
                                nc.tensor.matmul(
                                    o_ps[:, dh * 512 : dh * 512 + 512],
                                    lhsT,
                                    v_t[
                                        :,
                                        kt4 * XFW
                                        + dh * 512 : kt4 * XFW
                                        + dh * 512
                                        + 512,
                                    ],
                                    start=(kt4 == 0),
                                    stop=(kt4 == 3),
                                )
                            nc.tensor.matmul(
                                o_ps[:, 1024:1025],
                                lhsT,
                                v_t[:, kt4 * XFW + 1024 : kt4 * XFW + 1025],
                                start=(kt4 == 0),
                                stop=(kt4 == 3),
                            )
                        if kb == 2 * J + 1:
                            done = (J, *finalize_front(J, o_ps))
                        elif kb == 0:
                            nc.vector.tensor_copy(o_acc[J][:], o_ps[:, 0 : D + 1])
                        else:
                            nc.vector.tensor_add(
                                o_acc[J][:], o_acc[J][:], o_ps[:, 0 : D + 1]
                            )
                    return done

                # outproj(J) runs after pv_block(2J+3): >= one full ST+PV of
                # cover for the add/cast/transpose chain, and J=6 lands after
                # PV(15) keeping the PE warm through J=7's finalize.
                ready = {}
                for kb in range(NKB):
                    pts = st_block(kb)
                    done = pv_block(kb, pts)
                    if done is not None:
                        ready[done[0]] = done[1:]
                    J_out = (kb - 3) // 2
                    if kb >= 3 and kb % 2 == 1 and J_out in ready:
                        outproj_pe(J_out, *ready.pop(J_out))
                for J in sorted(ready):
                    outproj_pe(J, *ready.pop(J))

    nc.compile()
    _cache["nc"] = nc
    return nc


def _make_in_maps(inputs, w_query, w_key, w_value):
    bf = ml_dtypes.bfloat16
    x32 = inputs.astype(np.float32)
    xb = x32.astype(bf)
    xt = np.ascontiguousarray(xb.T)  # [D, S]
    # [16, 128, 8c*512] : xt_pre[kb, p, c*512+k] = x[kb*512+k, c*128+p]
    xt_pre = np.ascontiguousarray(
        xt.reshape(NCH, 128, NKB, 512).transpose(2, 1, 0, 3).reshape(NKB, 128, -1)
    )
    xf = np.zeros((S, XFW), dtype=bf)
    xf[:, 0:D] = xb
    xf[:, D] = np.float32(1.0)
    # [16, 128, 4c*1028] : xf_pre[kb, p, c*1028+w] = xf[kb*512+c*128+p, w]
    xf_pre = np.ascontiguousarray(
        xf.reshape(NKB, 4, 128, XFW).transpose(0, 2, 1, 3).reshape(NKB, 128, -1)
    )

    def fold(a):  # [1024, W] -> [128, 8*W] with chunk c at cols [c*W, (c+1)*W)
        W = a.shape[1]
        return np.ascontiguousarray(
            a.reshape(NCH, 128, W).transpose(1, 0, 2).reshape(128, NCH * W)
        )

    m = (w_query.astype(np.float32).T @ w_key.astype(np.float32)).astype(bf)
    m_pre = fold(m)
    wv_pre = fold(np.ascontiguousarray(w_value.T).astype(bf))

    kt_off = np.arange(8)[:, None, None] * 128 + np.arange(128)[None, :, None]
    in_maps = []
    for i in range(NC):
        xq = np.ascontiguousarray(xt[:, i::NC])  # [D, QPC]
        xq0 = fold(xq[:, 0:512])
        xq1 = fold(xq[:, 512:1024])
        q_off = np.arange(128)[None, None, :] * 8 + i
        mask = (kt_off <= q_off).astype(np.float32).astype(bf)  # [8,128,128]
        mask_pre = np.ascontiguousarray(
            mask.transpose(1, 0, 2).reshape(128, 8 * 128)
        )
        in_maps.append(
            {
                "m": m_pre,
                "xq0": xq0,
                "xq1": xq1,
                "xt": xt_pre,
                "xf": xf_pre,
                "wv": wv_pre,
                "masks": mask_pre,
            }
        )
    return in_maps


def run(inputs, w_query, w_key, w_value, trace=False):
    nc = _build()
    in_maps = _make_in_maps(inputs, w_query, w_key, w_value)
    res = bass_utils.run_bass_kernel_spmd(
        nc, in_maps, core_ids=list(range(NC)), trace=trace
    )
    full = np.empty((S, D), dtype=np.float32)
    for i in range(NC):
        full[i::NC] = res.results[i]["out"]
    return full, res


def kernel(inputs, w_query, w_key, w_value):
    inputs = np.asarray(inputs, dtype=np.float32)
    w_query = np.asarray(w_query, dtype=np.float32)
    w_key = np.asarray(w_key, dtype=np.float32)
    w_value = np.asarray(w_value, dtype=np.float32)
    full, _ = run(inputs, w_query, w_key, w_value, trace=False)
    return full
